# revision 7
# baseline (speedup 1.0000x reference)
"""BSQ quantizer kernel for Trainium2, data-parallel across 8 NeuronCores.

Math notes (vs the PyTorch/JAX reference):
  - zq = sign(zn) * Q where zn = z/||z|| and sign(zn) == sign(z); the
    straight-through estimator is an exact no-op in the forward pass (up to
    ~2^-23 relative rounding, far below any checking threshold).
  - The per-group softmax over 512 sign codes factorizes into per-bit
    Bernoullis: prob_d = prod_c sigmoid(4*Q*s_dc*zn_c).  So
    avg_prob[g] = mean_n softmaxA_n (x) softmaxB_n, with A = first 4 bits
    (16 values) and B = last 5 bits (32 values).  Each side is a product of
    per-bit probabilities (automatically normalized), and the outer product
    summed over samples is a matmul contracted over samples.
  - per-bit entropy: -(p log p + (1-p) log(1-p)) with p = sigmoid(+-x)
    equals softplus(x) - x*sigmoid(x), x = 4*Q*zn.
  - commit loss per sample: sum_d (Q*sign - zn_d)^2 = 2 - 2*Q*sum_d |zn_d|.
"""

import math

import numpy as np

import bass_rust
import concourse.bass as bass
import concourse.tile as tile
from concourse import mybir
from concourse.bass_utils import run_bass_kernel_spmd

AF = mybir.ActivationFunctionType
ALU = mybir.AluOpType
F32 = mybir.dt.float32
BF16 = mybir.dt.bfloat16
I32 = mybir.dt.int32

N_CORES = 8
B, L, D = 32, 4096, 18
N = B * L                    # 131072 samples
NC_SAMP = N // N_CORES       # 16384 per core
P = 128                      # partitions
T = NC_SAMP // P             # 128 t-chunks per core
Q = 1.0 / math.sqrt(D)


def _split_waits(nc, maxw=1):
    """walrus in this container only lowers one sync-wait per instruction;
    move excess waits onto freshly inserted NOPs just before the offender."""
    nsplit = 0
    for bb in nc.main_func.blocks:
        lst = bb.instructions
        i = 0
        while i < len(lst):
            ins = lst[i]
            si = ins.sync_info
            if si is not None and len(si.on_wait) > maxw:
                waits = list(si.on_wait)
                extra = waits[:-maxw]
                si.on_wait = waits[-maxw:]
                nops = []
                for j in range(0, len(extra), maxw):
                    nsplit += 1
                    nop = mybir.InstNoOp(name=f"I-waitsplit-{nsplit}")
                    nop.engine = ins.engine
                    nop.sync_info = bass_rust.SyncInfo(
                        on_wait=extra[j : j + maxw], on_update=[]
                    )
                    nops.append(nop)
                for k, nop in enumerate(nops):
                    lst.insert(i + k, nop)
                i += len(nops)
            i += 1
    return nsplit


def _ap(t, extra_offset, dims):
    """Build an AP over tile t's tensor: partition dim copied from t, free
    dims given explicitly as [step, count] (steps in elements)."""
    return bass.AP(tensor=t.tensor, offset=t.offset + extra_offset, ap=[t.ap[0]] + dims)


def _build_nc(split_waits=True):
    nc = bass.Bass()
    z_in = nc.dram_tensor("z", [P, D * T], F32, kind="ExternalInput")
    basis_in = nc.dram_tensor("basis", [D], F32, kind="ExternalInput")
    zq_out = nc.dram_tensor("zq", [P, D * T], F32, kind="ExternalOutput")
    idx_out = nc.dram_tensor("idx", [P, T], I32, kind="ExternalOutput")
    sums_out = nc.dram_tensor("sums", [P, 3], F32, kind="ExternalOutput")
    avgp_out = nc.dram_tensor("avgp", [32, 64], F32, kind="ExternalOutput")

    with tile.TileContext(nc) as tc:
        with (
            tc.tile_pool(name="main", bufs=1) as pool,
            tc.tile_pool(name="psum", bufs=1, space="PSUM") as psum,
        ):
            zt = pool.tile([P, D, T], F32)          # [p, d, t]
            nc.sync.dma_start(out=zt, in_=z_in[:, :].rearrange("p (d t) -> p d t", d=D))
            basis = pool.tile([P, D], F32)
            nc.sync.dma_start(
                out=basis,
                in_=bass.AP(tensor=basis_in, offset=0, ap=[[0, P], [1, D]]),
            )

            s1 = pool.tile([P, D, T], F32)          # scratch (z^2, sp, |zn|, silu)
            sums = pool.tile([P, 3], F32)
            # ---- natural_log_exp table set ----
            nc.scalar.activation(s1, zt, AF.Square)
            ssq = pool.tile([P, T], F32)
            nc.vector.tensor_reduce(
                out=ssq,
                in_=_ap(s1, 0, [[1, T], [T, D]]),   # iterate t-outer, d-inner
                axis=mybir.AxisListType.X,
                op=ALU.add,
            )
            ln_s = pool.tile([P, T], F32)
            nc.scalar.activation(ln_s, ssq, AF.Ln)
            rn = pool.tile([P, T], F32)
            nc.scalar.activation(rn, ln_s, AF.Exp, scale=-0.5)  # 1/sqrt(ssq)

            zn = pool.tile([P, D, T], F32)
            nc.vector.tensor_mul(zn, zt, _ap(rn, 0, [[0, D], [1, T]]))

            ex = pool.tile([P, D, T], F32)
            nc.scalar.activation(ex, zn, AF.Exp, scale=4.0 * Q)      # e^x
            nc.scalar.activation(                                     # softplus(x)
                s1, ex, AF.Ln, bias=1.0, accum_out=sums[:, 0:1]
            )
            nc.scalar.activation(                                     # 2Q|zn|
                s1, zn, AF.Abs, scale=2.0 * Q, accum_out=sums[:, 2:3]
            )
            # ---- sigmoid set ----
            pq = pool.tile([P, 2, D, T], BF16)      # [p, sign(0:-,1:+), d, t]
            nc.scalar.activation(pq[:, 1], zn, AF.Sigmoid, scale=4.0 * Q)
            # sum of zn*sigmoid(x) (x = 4Q*zn); host multiplies by 4Q
            nc.vector.tensor_mul(s1, zn, pq[:, 1])
            nc.vector.tensor_reduce(
                out=sums[:, 1:2],
                in_=s1,
                axis=mybir.AxisListType.XY,
                op=ALU.add,
            )
            nc.vector.tensor_scalar(
                out=pq[:, 0], in0=pq[:, 1], scalar1=-1.0, scalar2=1.0,
                op0=ALU.mult, op1=ALU.add,
            )

            # ---- sign path: zq and integer codes ----
            cmp = pool.tile([P, D, T], F32)         # 1.0 if z>0 else 0.0
            nc.vector.tensor_scalar(
                out=cmp, in0=zt, scalar1=0.0, scalar2=None, op0=ALU.is_gt
            )
            zqt = pool.tile([P, D, T], F32)
            nc.vector.tensor_scalar(
                out=zqt, in0=cmp, scalar1=2.0 * Q, scalar2=-Q,
                op0=ALU.mult, op1=ALU.add,
            )
            nc.sync.dma_start(
                out=zq_out[:, :].rearrange("p (d t) -> p d t", d=D), in_=zqt
            )
            prod = ex  # reuse: ex dead after softplus
            nc.vector.tensor_mul(prod, cmp, _ap(basis, 0, [[1, D], [0, T]]))
            idxf = pool.tile([P, T], F32)
            nc.vector.tensor_reduce(
                out=idxf,
                in_=_ap(prod, 0, [[1, T], [T, D]]),
                axis=mybir.AxisListType.X,
                op=ALU.add,
            )
            idxi = pool.tile([P, T], I32)
            nc.vector.tensor_copy(idxi, idxf)
            nc.sync.dma_start(out=idx_out[:, :], in_=idxi)

            # ---- Bernoulli product tree (bf16), per group ----
            # pq element (s, d, t) lives at s*D*T + d*T + t
            t16 = pool.tile([P, 2, 16, T], BF16)    # [p, g, i4, t]
            t32 = pool.tile([P, 2, 32, T], BF16)    # [p, g, j5, t]
            p01 = pool.tile([P, 4, T], BF16)
            p23 = pool.tile([P, 4, T], BF16)
            q45 = pool.tile([P, 4, T], BF16)
            q67 = pool.tile([P, 4, T], BF16)
            q678 = pool.tile([P, 8, T], BF16)

            def pair(dst, d_hi, d_lo, g):
                """dst[b_hi, b_lo, t] = pq[b_hi, d_hi, t] * pq[b_lo, d_lo, t]"""
                nc.vector.tensor_mul(
                    _ap(dst, 0, [[2 * T, 2], [T, 2], [1, T]]),
                    _ap(pq, (9 * g + d_hi) * T, [[D * T, 2], [0, 2], [1, T]]),
                    _ap(pq, (9 * g + d_lo) * T, [[0, 2], [D * T, 2], [1, T]]),
                )

            for g in range(2):
                pair(p01, 0, 1, g)
                pair(p23, 2, 3, g)
                # t16[g, (i01, i23), t] = p01[i01, t] * p23[i23, t]
                nc.vector.tensor_mul(
                    _ap(t16, g * 16 * T, [[4 * T, 4], [T, 4], [1, T]]),
                    _ap(p01, 0, [[T, 4], [0, 4], [1, T]]),
                    _ap(p23, 0, [[0, 4], [T, 4], [1, T]]),
                )
                pair(q45, 4, 5, g)
                pair(q67, 6, 7, g)
                # q678[(i67, b8), t] = q67[i67, t] * pq[b8, d=8, t]
                nc.vector.tensor_mul(
                    _ap(q678, 0, [[2 * T, 4], [T, 2], [1, T]]),
                    _ap(q67, 0, [[T, 4], [0, 2], [1, T]]),
                    _ap(pq, (9 * g + 8) * T, [[0, 4], [D * T, 2], [1, T]]),
                )
                # t32[g, (i45, i678), t] = q45[i45, t] * q678[i678, t]
                nc.vector.tensor_mul(
                    _ap(t32, g * 32 * T, [[8 * T, 4], [T, 8], [1, T]]),
                    _ap(q45, 0, [[T, 4], [0, 8], [1, T]]),
                    _ap(q678, 0, [[0, 4], [T, 8], [1, T]]),
                )

            # ---- avg_prob partial: sum_n a (x) b via PE, contract over samples
            ps = psum.tile([32, 64], F32)
            for t in range(T):
                nc.tensor.matmul(
                    ps,
                    _ap(t16, t, [[16 * T, 2], [T, 16]]),   # [K=128, (g,i)=32]
                    _ap(t32, t, [[32 * T, 2], [T, 32]]),   # [K=128, (g,j)=64]
                    start=(t == 0),
                    stop=(t == T - 1),
                )
            avg_sb = pool.tile([32, 64], F32)
            nc.scalar.copy(avg_sb, ps)
            nc.sync.dma_start(out=avgp_out[:, :], in_=avg_sb)
            nc.sync.dma_start(out=sums_out[:, :], in_=sums)

    if split_waits:
        _split_waits(nc)
    return nc


_NC_CACHE = None


def _get_nc():
    global _NC_CACHE
    if _NC_CACHE is None:
        _NC_CACHE = _build_nc()
    return _NC_CACHE


_BASIS = (2.0 ** np.arange(D - 1, -1, -1)).astype(np.float32)


def _shard(z):
    """z [B, L, D] f32 -> list of per-core input dicts (device layout)."""
    zf = np.ascontiguousarray(z, dtype=np.float32).reshape(N, D)
    ins = []
    for c in range(N_CORES):
        zc = zf[c * NC_SAMP : (c + 1) * NC_SAMP]          # [16384, 18]
        zdev = zc.reshape(T, P, D).transpose(1, 2, 0)      # [p, d, t]
        ins.append(
            {
                "z": np.ascontiguousarray(zdev).reshape(P, D * T),
                "basis": _BASIS,
            }
        )
    return ins


def _unshard(results):
    zq = np.empty((N, D), np.float32)
    idx = np.empty((N,), np.int32)
    sp_t = sw_t = cm_t = 0.0
    avg = np.zeros((32, 64), np.float64)
    for c, r in enumerate(results):
        zq_dev = r["zq"].reshape(P, D, T)
        zq[c * NC_SAMP : (c + 1) * NC_SAMP] = (
            zq_dev.transpose(2, 0, 1).reshape(NC_SAMP, D)
        )
        idx[c * NC_SAMP : (c + 1) * NC_SAMP] = r["idx"].T.reshape(NC_SAMP)
        s = r["sums"].astype(np.float64)
        sp_t += s[:, 0].sum()
        sw_t += s[:, 1].sum()
        cm_t += s[:, 2].sum()
        avg += r["avgp"].astype(np.float64)

    persample = (sp_t - 4.0 * Q * sw_t) / N
    commit = 2.0 - cm_t / N
    ap = np.empty((2, 512), np.float64)
    for g in range(2):
        ap[g] = (avg[g * 16 : (g + 1) * 16, g * 32 : (g + 1) * 32] / N).reshape(512)
    cb_ent = float(-(ap * np.log(ap + 1e-8)).sum())
    total = commit + persample - cb_ent

    return (
        zq.reshape(B, L, D),
        np.float32(total),
        np.float32(persample),
        np.float32(cb_ent),
        np.float32(commit),
        idx.reshape(B, L),
    )


def kernel(z):
    nc = _get_nc()
    ins = _shard(np.asarray(z))
    res = run_bass_kernel_spmd(nc, ins, core_ids=list(range(N_CORES)))
    return _unshard(res.results)
